# revision 62
# baseline (speedup 1.0000x reference)
"""Trainium2 Bass kernel for the 2-player masked LSTM scan.

Reference semantics (T=128 steps, B=256 batch, C=1024 in, H=1024 hidden):
  per step t, batch b: the active player's (c,h) (selected by main[t,b]) runs
  one LSTM cell z = x@Wi + h@Wh + b with fused i,f,g,o gates; the result is
  written back only to the active player's state, and both players' states are
  zeroed where done[t,b].

Algorithmic structure: done/main are inputs, so the dependency structure is
known on the host.  Each (b, segment, player) triple is an independent chain;
position depth d depends only on depth d-1.  Chains sorted by length and
round-robined over the 8 cores turn the scan into D~9 dense waves.

Device schedule (zero-init fast path):
 - phase A computes zx = x@Wi + b for all depth>=1 positions (the small "zx
   tiles", emitted FIRST so every wave's zx is ready early), then the depth-0
   positions fused with their full gate math ("w0 tiles"; c == 0 there, so the
   f-gate matmuls are skipped entirely -- 25% less PE work on those tiles).
 - phase B waves run h@Wh in fp8 e4m3 with DoubleRow perf mode (2 k-subtiles
   per matmul, 2x PE throughput; rel-err contribution measured ~0.004), the
   bf16 zx injected into PSUM via an identity matmul so the ACT engine reads
   gates straight from PSUM.  Wave jobs are interleaved into the phase-A tile
   stream: single-m-tile tail waves get a w0 "filler" tile emitted after them
   so the PE stays busy during the serial gate-math/transpose latency.
 - inter-wave state: h^T via per-half xbar transposes (bf16) + fp8 cast into
   per-wave SBUF tiles; c stays in SBUF for single-tile consumers and round-
   trips DRAM for the big early waves.
"""

import os
import sys

sys.path.insert(0, "/opt/trn_rl_repo")

import numpy as np
import ml_dtypes

import concourse.bass as bass
import concourse.tile as tile
from concourse import bacc, mybir
from concourse.bass_utils import run_bass_kernel_spmd

BF16 = ml_dtypes.bfloat16
F8 = ml_dtypes.float8_e4m3
AF = mybir.ActivationFunctionType
DT = mybir.dt
DR = mybir.MatmulPerfMode.DoubleRow

NCORES = 8
H = 1024
CIN = 1024
G = 4 * H
KT = CIN // 128  # 8 k-tiles for both contractions

# i-gate of the x@Wi matmul in fp8 DoubleRow (error budget trade: measured
# rel err 0.0154 vs 0.0067 all-bf16, for ~32us less HW time).
FP8_I = bool(int(os.environ.get("KERNEL_FP8_I", "1")))
XSC = 8.0    # fp8 scale for x
WSC = 32.0   # fp8 scale for Wi i-gate columns


# ---------------------------------------------------------------------------
# Host-side schedule construction
# ---------------------------------------------------------------------------

def _build_schedule(done, main, T, B):
    """Chain decomposition of the (t, b) grid."""
    done2 = done.reshape(T, B).astype(bool)
    main2 = main.reshape(T, B).astype(bool)

    seg = np.zeros((T, B), np.int64)
    if T > 1:
        seg[1:] = np.cumsum(done2[:-1], axis=0)
    player = main2.astype(np.int64)
    key = (np.arange(B)[None, :] * (T + 1) + seg) * 2 + player  # [T, B]
    flat_key = key.reshape(-1)  # position p = t*B + b
    order = np.argsort(flat_key, kind="stable")  # chain-major, t-ascending
    sorted_keys = flat_key[order]
    uk, first_idx, inv = np.unique(sorted_keys, return_index=True, return_inverse=True)
    chain_len = np.diff(np.append(first_idx, len(sorted_keys)))
    npos = T * B

    depth = np.empty(npos, np.int64)
    depth[order] = np.arange(npos) - first_idx[inv]
    chain_id = np.empty(npos, np.int64)
    chain_id[order] = inv

    n_chains = len(uk)
    chain_b = uk // (2 * (T + 1))
    chain_seg = (uk // 2) % (T + 1)
    chain_player = uk % 2

    chain_order = np.argsort(-chain_len, kind="stable")
    rank_of_chain = np.empty(n_chains, np.int64)
    rank_of_chain[chain_order] = np.arange(n_chains)
    core_of_chain = (rank_of_chain % NCORES).astype(np.int64)
    core_rank = rank_of_chain // NCORES

    D = int(chain_len.max())
    lens_sorted = np.sort(chain_len)
    N_d = np.array([n_chains - np.searchsorted(lens_sorted, d, side="right")
                    for d in range(D)], np.int64)
    U = np.ceil(N_d / NCORES).astype(np.int64)      # uniform per-core wave rows
    M = np.ceil(U / 128).astype(np.int64)           # padded wave m-tiles
    V = np.concatenate([[0], np.cumsum(U)])          # packed row offsets
    P = np.concatenate([[0], np.cumsum(M * 128)])    # padded row offsets

    return dict(
        depth=depth, chain_id=chain_id, core_of_chain=core_of_chain,
        core_rank=core_rank, chain_b=chain_b, chain_seg=chain_seg,
        chain_player=chain_player, D=D, U=U, M=M, V=V, P=P,
    )


def _zx_layout(D, U, M, V, zero_init):
    """zx-region wave offsets + m-tile count.

    Waves are packed largest-first with the minimum-padding wave last, so
    every wave's padded 128-row-aligned reads land inside the region with
    minimal total padding (only the last-placed wave's padding is exposed).
    Returns (Zt, zoff) with zoff[d] the region row offset of wave d.
    """
    dz = 1 if zero_init else 0
    if D <= dz:
        return 0, {}
    waves = list(range(dz, D))

    def layout(order):
        zoff = {}
        acc = 0
        for d in order:
            zoff[d] = acc
            acc += int(U[d])
        need = max(zoff[d] + int(M[d]) * 128 for d in order)
        return zoff, need

    pad = {d: int(M[d]) * 128 - int(U[d]) for d in waves}
    last = min(waves, key=lambda d: pad[d])
    cand = sorted(waves, key=lambda d: -int(U[d]))
    cand.remove(last)
    cand.append(last)
    best_off, best_need = layout(cand)
    off2, need2 = layout(waves)  # plain ascending fallback
    if need2 < best_need:
        best_off, best_need = off2, need2
    return (best_need + 127) // 128, best_off


def _prep_inputs(x, c1, h1, c2, h2, Wi, Wh, b, done, main):
    """Build per-core device input arrays + output scatter indices."""
    B = c1.shape[0]
    T = x.shape[0] // B
    sch = _build_schedule(np.asarray(done), np.asarray(main), T, B)
    D, U, M, V, P = sch["D"], sch["U"], sch["M"], sch["V"], sch["P"]

    zero_init = not (np.any(c1) or np.any(h1) or np.any(c2) or np.any(h2))
    dz = 1 if zero_init else 0
    Zt, zoff = _zx_layout(D, U, M, V, zero_init)
    M0 = int(M[0])
    NT = Zt + (M0 if zero_init else 0)

    depth = sch["depth"]
    chain_id = sch["chain_id"]
    core_pos = sch["core_of_chain"][chain_id]
    rank = sch["core_rank"][chain_id]
    zoff_arr = np.zeros(D + 1, np.int64)
    for d, o in zoff.items():
        zoff_arr[d] = o
    if zero_init:
        xrow = np.where(depth == 0, Zt * 128 + rank, zoff_arr[depth] + rank)
    else:
        xrow = zoff_arr[depth] + rank
    padded_row = P[depth] + rank

    x = np.ascontiguousarray(np.asarray(x, np.float32))
    xt_blocks = []
    xt8_blocks = []
    for c in range(NCORES):
        sel = core_pos == c
        Xp = np.zeros((NT * 128, CIN), np.float32)
        Xp[xrow[sel]] = x[sel]
        # lhsT block layout: [mt, p, k*128 + m] = Xp[mt*128+m, k*128+p]
        xt = Xp.reshape(NT, 128, KT, 128).transpose(0, 3, 2, 1).reshape(NT, 128, CIN)
        xt_blocks.append(np.ascontiguousarray(xt.astype(BF16)))
        if FP8_I:
            xt8_blocks.append(np.ascontiguousarray(
                np.clip(xt * XSC, -240, 240).astype(F8)))

    Wi_f32 = np.asarray(Wi, np.float32)
    if FP8_I:
        Wi_l = np.ascontiguousarray(
            Wi_f32[:, H:].reshape(CIN, 3 * H).reshape(KT, 128, 3 * H).astype(BF16))
        Wi_i8 = np.ascontiguousarray(
            np.clip(Wi_f32[:, :H] * WSC, -240, 240).reshape(KT, 128, H).astype(F8))
    else:
        Wi_l = np.ascontiguousarray(Wi_f32.reshape(KT, 128, G).astype(BF16))
        Wi_i8 = None
    Wh8 = np.ascontiguousarray(
        np.asarray(Wh, np.float32).reshape(KT, 128, G).astype(F8))
    bbc = np.ascontiguousarray(
        np.broadcast_to(np.asarray(b, np.float32)[None, :], (128, G)).astype(BF16))
    ident = np.ascontiguousarray(np.eye(128, dtype=np.float32).astype(BF16))

    ht0_blocks = [None] * NCORES
    c0_blocks = [None] * NCORES
    if not zero_init:
        h1 = np.asarray(h1, np.float32); h2 = np.asarray(h2, np.float32)
        c1 = np.asarray(c1, np.float32); c2 = np.asarray(c2, np.float32)
        hin = np.where(sch["chain_player"][:, None] > 0, h1[sch["chain_b"]],
                       h2[sch["chain_b"]])
        cin_ = np.where(sch["chain_player"][:, None] > 0, c1[sch["chain_b"]],
                        c2[sch["chain_b"]])
        live = sch["chain_seg"] == 0
        hin = np.where(live[:, None], hin, 0.0)
        cin_ = np.where(live[:, None], cin_, 0.0)
        for c in range(NCORES):
            selc = sch["core_of_chain"] == c
            rows = sch["core_rank"][selc]
            Hp = np.zeros((M0 * 128, H), np.float32)
            Cp = np.zeros((M0 * 128, H), np.float32)
            Hp[rows] = hin[selc]
            Cp[rows] = cin_[selc]
            # transposed fp8 layout for lhsT: [p, k, col] = Hp[col, k*128+p]
            ht0 = Hp.reshape(M0 * 128, KT, 128).transpose(2, 1, 0)
            ht0_blocks[c] = np.ascontiguousarray(ht0.astype(F8))
            c0_blocks[c] = np.ascontiguousarray(Cp)

    return dict(
        sch=sch, zero_init=zero_init, Zt=Zt, NT=NT,
        xt_blocks=xt_blocks, xt8_blocks=xt8_blocks, Wi_l=Wi_l, Wi_i8=Wi_i8,
        Wh8=Wh8, bbc=bbc, ident=ident,
        ht0_blocks=ht0_blocks, c0_blocks=c0_blocks,
        core_pos=core_pos, padded_row=padded_row, T=T, B=B,
    )


# ---------------------------------------------------------------------------
# Device program
# ---------------------------------------------------------------------------

def _build_program(D, U, M, V, P, zero_init):
    nc = bacc.Bacc("TRN2", target_bir_lowering=False, debug=False)

    M_ = [int(m) for m in M]
    P_ = [int(p) for p in P]
    dz = 1 if zero_init else 0
    M0 = M_[0]
    Zt, Vz = _zx_layout(D, U, M, V, zero_init)
    NT = Zt + (M0 if zero_init else 0)
    Ptot = P_[D]
    have_waves = D > dz

    WIW = 3 * H if FP8_I else G  # bf16 Wi width (i-gate split out when fp8)
    xt_d = nc.dram_tensor("xt", [NT, 128, CIN], DT.bfloat16, kind="ExternalInput")
    wi_d = nc.dram_tensor("wi", [KT, 128, WIW], DT.bfloat16, kind="ExternalInput")
    if FP8_I:
        xt8_d = nc.dram_tensor("xt8", [NT, 128, CIN], DT.float8e4,
                               kind="ExternalInput")
        wi8_d = nc.dram_tensor("wi8", [KT, 128, H], DT.float8e4,
                               kind="ExternalInput")
    bbc_d = nc.dram_tensor("bbc", [128, G], DT.bfloat16, kind="ExternalInput")
    id_d = nc.dram_tensor("ident", [128, 128], DT.bfloat16, kind="ExternalInput")
    y_d = nc.dram_tensor("y", [Ptot, H], DT.bfloat16, kind="ExternalOutput")
    if have_waves:
        wh_d = nc.dram_tensor("wh8", [KT, 128, G], DT.float8e4,
                              kind="ExternalInput")
        zx_d = nc.dram_tensor("zx", [Zt * 128, 2, 2048], DT.bfloat16,
                              kind="Internal")
        zx_ap = zx_d.ap()
    # c-state DRAM regions for waves whose consumer wave has >1 m-tile
    cs_off = {}
    acc = 0
    for d in range(D - 1):
        if M_[d + 1] > 1:
            cs_off[d] = acc
            acc += M_[d + 1] * 128
    if acc:
        cs_d = nc.dram_tensor("cstate", [acc, H], DT.float32, kind="Internal")
        cs_ap = cs_d.ap()
    if not zero_init:
        ht0_d = nc.dram_tensor("ht0", [128, KT, M0 * 128], DT.float8e4,
                               kind="ExternalInput")
        c0_d = nc.dram_tensor("c0", [M0 * 128, H], DT.float32,
                              kind="ExternalInput")

    xt_ap = xt_d.ap()
    y_ap = y_d.ap()

    with tile.TileContext(nc) as tc:
        from contextlib import ExitStack
        with ExitStack() as es:
            const = es.enter_context(tc.tile_pool(name="const", bufs=1))
            wp = es.enter_context(tc.tile_pool(name="weights", bufs=1))
            xtP = es.enter_context(tc.tile_pool(name="xtP", bufs=3))
            zbP = es.enter_context(tc.tile_pool(name="zbP", bufs=2))
            zxP = es.enter_context(tc.tile_pool(name="zxP", bufs=6))
            gP = es.enter_context(tc.tile_pool(name="gP", bufs=1))
            fP = es.enter_context(tc.tile_pool(name="fP", bufs=2))
            sP = es.enter_context(tc.tile_pool(name="sP", bufs=2))
            hTP = es.enter_context(tc.tile_pool(name="hTP", bufs=1))
            cP = es.enter_context(tc.tile_pool(name="cP", bufs=2))
            psum = es.enter_context(tc.tile_pool(name="psum", bufs=2,
                                                 space="PSUM"))

            # ---- weights / constants (emission order = DMA priority) ----
            # k=0,1 immediately (first matmuls); k=2..7 and wh8 staged after
            # the first jobs so the bulk loads don't steal HBM bandwidth from
            # the startup-critical xt/wi-k0 transfers.
            wi_sb = wp.tile([128, KT, WIW], DT.bfloat16, tag="wi")
            # first k-slices split in two so they spread across DMA engines
            nc.gpsimd.dma_start(out=wi_sb[0:64, 0:1, :], in_=wi_d.ap()[0][0:64, :])
            nc.gpsimd.dma_start(out=wi_sb[64:128, 0:1, :],
                                in_=wi_d.ap()[0][64:128, :])
            nc.sync.dma_start(out=wi_sb[:, 1:2, :], in_=wi_d.ap()[1])
            if FP8_I:
                wi8_sb = wp.tile([128, KT, H], DT.float8e4, tag="wi8")
                for k in range(KT):
                    nc.sync.dma_start(out=wi8_sb[:, k:k + 1, :],
                                      in_=wi8_d.ap()[k])
            if have_waves:
                wh8_sb = wp.tile([128, KT, G], DT.float8e4, tag="wh8")
            ident_sb = const.tile([128, 128], DT.bfloat16, tag="ident")
            bbc_sb = const.tile([128, G], DT.bfloat16, tag="bbc")

            hT8 = {}

            def hT_target(dn):
                """hT tile for consumer wave dn (rotating tag for 1-tile waves)."""
                if dn not in hT8:
                    one = M_[dn] == 1
                    hT8[dn] = hTP.tile(
                        [128, KT, M_[dn] * 128], DT.float8e4,
                        tag="hTt" if one else f"hT{dn}", name=f"hT{dn}",
                        bufs=2 if one else 1)
                return hT8[dn]
            if not zero_init:
                ht0_sb = wp.tile([128, KT, M0 * 128], DT.float8e4, tag="ht0")
                nc.gpsimd.dma_start(out=ht0_sb[:], in_=ht0_d.ap()[:])

            # gate tiles
            gi = gP.tile([128, H], DT.bfloat16, tag="gi", name="gi")
            gf = gP.tile([128, H], DT.bfloat16, tag="gf", name="gf")
            gg = gP.tile([128, H], DT.bfloat16, tag="gg", name="gg")
            go = gP.tile([128, H], DT.bfloat16, tag="go", name="go")

            keep_state = {}  # wave d -> SBUF fp32 c tile (single-tile consumer)

            def mm_phaseA(xt_sb, xt8_sb, half, skip_f):
                pt = {}
                for g in range(4):
                    if skip_f and g == 1:
                        continue
                    pt[g] = psum.tile([128, 512], DT.float32, tag=f"ph{g}",
                                      name=f"ph{g}")
                for k in range(KT):
                    for g in pt:
                        if FP8_I and g == 0:
                            continue
                        col0 = ((g - 1) * H if FP8_I else g * H) + half * 512
                        nc.tensor.matmul(
                            pt[g][:], lhsT=xt_sb[:, k:k + 1, :],
                            rhs=wi_sb[:, k:k + 1, col0:col0 + 512],
                            start=(k == 0), stop=(k == KT - 1),
                            skip_group_check=True)
                if FP8_I:
                    for kp in range(KT // 2):
                        nc.tensor.matmul(
                            pt[0][:], lhsT=xt8_sb[:, 2 * kp:2 * kp + 2, :],
                            rhs=wi8_sb[:, 2 * kp:2 * kp + 2,
                                       half * 512:half * 512 + 512],
                            start=(kp == 0), stop=(kp == KT // 2 - 1),
                            perf_mode=DR, skip_group_check=True)
                return pt

            def drain(pt, half):
                """PSUM -> SBUF bf16 with bias add (DVE; GPSIMD can't read PSUM)."""
                zbs = {}
                for g, p in pt.items():
                    col0 = g * H + half * 512
                    zb = zbP.tile([128, 512], DT.bfloat16, tag=f"zb{g}",
                                  name=f"zb{g}")
                    if FP8_I and g == 0:
                        nc.vector.scalar_tensor_tensor(
                            zb[:], p[:], 1.0 / (XSC * WSC),
                            bbc_sb[:, col0:col0 + 512],
                            mybir.AluOpType.mult, mybir.AluOpType.add)
                    else:
                        nc.vector.tensor_add(zb[:], p[:],
                                             bbc_sb[:, col0:col0 + 512])
                    zbs[g] = zb
                return zbs

            def load_tile(t, first):
                xt_sb = xtP.tile([128, KT, 128], DT.bfloat16, tag="xt",
                                 name="xt")
                nc.gpsimd.dma_start(out=xt_sb[:], in_=xt_ap[t])
                xt8_sb = None
                if FP8_I:
                    xt8_sb = xtP.tile([128, KT, 128], DT.float8e4, tag="xt8",
                                      name="xt8")
                    nc.gpsimd.dma_start(out=xt8_sb[:], in_=xt8_d.ap()[t])
                if first == 0:
                    nc.sync.dma_start(out=bbc_sb[:], in_=bbc_d.ap()[:])
                    nc.sync.dma_start(out=ident_sb[:], in_=id_d.ap()[:])
                    for k in range(2, KT):
                        nc.sync.dma_start(out=wi_sb[:, k:k + 1, :],
                                          in_=wi_d.ap()[k])
                elif first == 1 and have_waves:
                    for k in range(KT):
                        nc.sync.dma_start(out=wh8_sb[:, k:k + 1, :],
                                          in_=wh_d.ap()[k])
                return xt_sb, xt8_sb

            njob = [0]

            def zx_job(t):
                xt_sb, xt8_sb = load_tile(t, njob[0])
                for half in range(2):
                    pt = mm_phaseA(xt_sb, xt8_sb, half, skip_f=False)
                    zbs = drain(pt, half)
                    for g in range(4):
                        nc.sync.dma_start(
                            out=zx_ap[t * 128:(t + 1) * 128, half:half + 1,
                                      g * 512:(g + 1) * 512],
                            in_=zbs[g][:])

            def w0_job(j):
                """Fused depth-0 tile (zero-init: c == 0, f gate skipped)."""
                feeds = have_waves and j < M_[1]
                xt_sb, xt8_sb = load_tile(Zt + j, njob[0])

                if feeds and M_[1] == 1:
                    ncv = sP.tile([128, H], DT.float32, tag="keepT",
                                  name="ncv_keep", bufs=2)
                    keep_state[0] = ncv
                else:
                    ncv = sP.tile([128, H], DT.float32, tag="ncv", name="ncv")
                nh = sP.tile([128, H], DT.bfloat16, tag="nh", name="nh")
                if feeds:
                    hTb = sP.tile([128, KT, 128], DT.bfloat16, tag="hTb",
                                  name="hTb")
                for half in range(2):
                    blk = slice(half * 512, half * 512 + 512)
                    pt = mm_phaseA(xt_sb, xt8_sb, half, skip_f=True)
                    zbs = drain(pt, half)
                    nc.scalar.activation(gi[:, blk], zbs[0][:], AF.Sigmoid)
                    nc.scalar.activation(gg[:, blk], zbs[2][:], AF.Tanh)
                    nc.vector.tensor_mul(ncv[:, blk], gi[:, blk], gg[:, blk])
                    nc.scalar.activation(go[:, blk], zbs[3][:], AF.Sigmoid)
                    tancb = fP.tile([128, 512], DT.float32, tag="tanc",
                                    name="tanc")
                    nc.scalar.activation(tancb[:], ncv[:, blk], AF.Tanh)
                    nc.vector.tensor_mul(nh[:, blk], go[:, blk], tancb[:])
                    if feeds:
                        nc.sync.dma_start_transpose(
                            out=hTb[:, half * 4:(half + 1) * 4, :],
                            in_=nh[:, blk])
                        nc.scalar.activation(
                            hT_target(1)[:, half * 4:(half + 1) * 4,
                                         j * 128:(j + 1) * 128],
                            hTb[:, half * 4:(half + 1) * 4, :], AF.Copy)
                    nc.sync.dma_start(out=y_ap[j * 128:(j + 1) * 128, blk],
                                      in_=nh[:, blk])
                if feeds and M_[1] > 1:
                    nc.sync.dma_start(
                        out=cs_ap[cs_off[0] + j * 128:cs_off[0] + (j + 1) * 128, :],
                        in_=ncv[:])

            def prefetch_zx(d, j):
                """Issue a wave tile's zx reads, split across DMA engines."""
                zxh = []
                r0 = Vz[d] + j * 128
                for half in range(2):
                    zt = zxP.tile([128, 2048], DT.bfloat16, tag="zxh",
                                  name="zxh")
                    for q in range(2):
                        nc.gpsimd.dma_start(
                            out=zt[q * 64:(q + 1) * 64, :],
                            in_=zx_ap[r0 + q * 64:r0 + (q + 1) * 64,
                                      half:half + 1, :])
                    zxh.append(zt)
                return zxh

            def prefetch_c(d, j):
                """Issue a wave tile's c-state read (called 1 job early)."""
                if d == 0 and not zero_init:
                    src = c0_d.ap()
                    base = j * 128
                elif M_[d] > 1:
                    src = cs_ap
                    base = cs_off[d - 1] + j * 128
                else:
                    return None
                c_src = cP.tile([128, H], DT.float32, tag="c", name="c_sb")
                for q in range(2):
                    nc.sync.dma_start(
                        out=c_src[q * 64:(q + 1) * 64, :],
                        in_=src[base + q * 64:base + (q + 1) * 64, :])
                return c_src

            def wv_job(d, j, zxh, c_src):
                feeds = (d + 1 < D) and (j < M_[d + 1])
                if c_src is None:
                    c_src = keep_state[d - 1]
                # lhsT source
                hsrc = ht0_sb if (d == 0 and not zero_init) else hT8[d]

                if feeds and M_[d + 1] == 1:
                    ncv = sP.tile([128, H], DT.float32, tag="keepT",
                                  name="ncv_keep", bufs=2)
                    keep_state[d] = ncv
                else:
                    ncv = sP.tile([128, H], DT.float32, tag="ncv", name="ncv")
                nh = sP.tile([128, H], DT.bfloat16, tag="nh", name="nh")
                if feeds:
                    hTb = sP.tile([128, KT, 128], DT.bfloat16, tag="hTb",
                                  name="hTb")
                for half in range(2):
                    blk = slice(half * 512, half * 512 + 512)
                    pt = []
                    for g in range(4):
                        pt.append(psum.tile([128, 512], DT.float32,
                                            tag=f"ph{g}", name=f"ph{g}"))
                    for kp in range(KT // 2):
                        for g in range(4):
                            col0 = g * H + half * 512
                            nc.tensor.matmul(
                                pt[g][:],
                                lhsT=hsrc[:, 2 * kp:2 * kp + 2,
                                          j * 128:(j + 1) * 128],
                                rhs=wh8_sb[:, 2 * kp:2 * kp + 2,
                                           col0:col0 + 512],
                                start=(kp == 0), stop=False,
                                perf_mode=DR, skip_group_check=True)
                    for g in range(4):
                        nc.tensor.matmul(
                            pt[g][:], lhsT=ident_sb[:],
                            rhs=zxh[half][:, g * 512:(g + 1) * 512],
                            start=False, stop=True, skip_group_check=True)
                    # gates straight from PSUM (zx inject carries the bias)
                    nc.scalar.activation(gi[:, blk], pt[0][:], AF.Sigmoid)
                    nc.scalar.activation(gg[:, blk], pt[2][:], AF.Tanh)
                    m1b = fP.tile([128, 512], DT.float32, tag="m1", name="m1")
                    nc.vector.tensor_mul(m1b[:], gi[:, blk], gg[:, blk])
                    nc.scalar.activation(gf[:, blk], pt[1][:], AF.Sigmoid)
                    t1b = fP.tile([128, 512], DT.float32, tag="t1", name="t1")
                    nc.vector.tensor_mul(t1b[:], gf[:, blk], c_src[:, blk])
                    nc.vector.tensor_add(ncv[:, blk], t1b[:], m1b[:])
                    nc.scalar.activation(go[:, blk], pt[3][:], AF.Sigmoid)
                    tancb = fP.tile([128, 512], DT.float32, tag="tanc",
                                    name="tanc")
                    nc.scalar.activation(tancb[:], ncv[:, blk], AF.Tanh)
                    nc.vector.tensor_mul(nh[:, blk], go[:, blk], tancb[:])
                    if feeds:
                        nc.sync.dma_start_transpose(
                            out=hTb[:, half * 4:(half + 1) * 4, :],
                            in_=nh[:, blk])
                        nc.scalar.activation(
                            hT_target(d + 1)[:, half * 4:(half + 1) * 4,
                                             j * 128:(j + 1) * 128],
                            hTb[:, half * 4:(half + 1) * 4, :], AF.Copy)
                    nc.sync.dma_start(
                        out=y_ap[P_[d] + j * 128:P_[d] + (j + 1) * 128, blk],
                        in_=nh[:, blk])
                if feeds and M_[d + 1] > 1:
                    nc.sync.dma_start(
                        out=cs_ap[cs_off[d] + j * 128:
                                  cs_off[d] + (j + 1) * 128, :],
                        in_=ncv[:])

            # ---- job order ----
            jobs = []
            if zero_init:
                jobs += [("zx", t) for t in range(Zt)]
                head = min(M_[1], M0) if have_waves else 0
                jobs += [("w0", j) for j in range(head)]
                from collections import deque
                fill = deque(range(head, M0))
                for d in range(1, D):
                    for j in range(M_[d]):
                        jobs.append(("wv", d, j))
                    if d + 1 < D and fill:
                        jobs.append(("w0", fill.popleft()))
                while fill:
                    jobs.append(("w0", fill.popleft()))
            else:
                jobs += [("zx", t) for t in range(Zt)]
                for d in range(D):
                    for j in range(M_[d]):
                        jobs.append(("wv", d, j))

            # Dispatch with 1-job DMA prefetch for wave tiles so their zx/c
            # reads are issued (and in flight) a full job ahead of the PE.
            pre_zx = {}
            pre_c = {}
            for idx, job in enumerate(jobs):
                for nxt in (idx + 1, idx + 2, idx + 3):
                    if nxt < len(jobs) and jobs[nxt][0] == "wv" \
                            and nxt not in pre_zx:
                        pre_zx[nxt] = prefetch_zx(jobs[nxt][1], jobs[nxt][2])
                nxt = idx + 1
                if nxt < len(jobs) and jobs[nxt][0] == "wv" \
                        and nxt not in pre_c:
                    pre_c[nxt] = prefetch_c(jobs[nxt][1], jobs[nxt][2])
                if job[0] == "zx":
                    zx_job(job[1])
                    njob[0] += 1
                elif job[0] == "w0":
                    w0_job(job[1])
                    njob[0] += 1
                else:
                    if idx not in pre_zx:
                        pre_zx[idx] = prefetch_zx(job[1], job[2])
                    if idx not in pre_c:
                        pre_c[idx] = prefetch_c(job[1], job[2])
                    wv_job(job[1], job[2], pre_zx.pop(idx), pre_c.pop(idx))

    nc.compile()
    return nc


# ---------------------------------------------------------------------------
# Entry point
# ---------------------------------------------------------------------------

_PROGRAM_CACHE = {}


def _run(inputs, trace=False):
    prep = _prep_inputs(**inputs)
    sch = prep["sch"]
    D, U, M, V, P = sch["D"], sch["U"], sch["M"], sch["V"], sch["P"]

    in_maps = []
    for c in range(NCORES):
        m = {
            "xt": prep["xt_blocks"][c],
            "wi": prep["Wi_l"],
            "bbc": prep["bbc"],
            "ident": prep["ident"],
        }
        if FP8_I:
            m["xt8"] = prep["xt8_blocks"][c]
            m["wi8"] = prep["Wi_i8"]
        if D > (1 if prep["zero_init"] else 0):
            m["wh8"] = prep["Wh8"]
        if not prep["zero_init"]:
            m["ht0"] = prep["ht0_blocks"][c]
            m["c0"] = prep["c0_blocks"][c]
        in_maps.append(m)

    # Retry ladder: rare transient device errors have been observed on the
    # shared terminal; retry a couple of times.
    import time as _time
    res = None
    last_err = None
    for attempt in range(3):
        key = (D, tuple(int(v) for v in M), tuple(int(v) for v in U),
               prep["zero_init"], FP8_I)
        try:
            if key not in _PROGRAM_CACHE:
                _PROGRAM_CACHE[key] = _build_program(
                    D, U, M, V, P, prep["zero_init"])
            nc = _PROGRAM_CACHE[key]
            res = run_bass_kernel_spmd(nc, in_maps,
                                       core_ids=list(range(NCORES)),
                                       trace=trace)
            break
        except Exception as e:  # noqa: BLE001 - retry on device hiccups
            last_err = e
            sys.stderr.write(f"kernel attempt {attempt} failed: {e!r}\n")
            trace = False  # profiling hook may be wedged; drop it on retry
            _time.sleep(2.0)
    if res is None:
        raise last_err

    T, B = prep["T"], prep["B"]
    y_full = np.empty((T * B, H), np.float32)
    core_pos = prep["core_pos"]
    padded_row = prep["padded_row"]
    for c in range(NCORES):
        sel = core_pos == c
        y_full[sel] = res.results[c]["y"][padded_row[sel]].astype(np.float32)
    return y_full, res


def kernel(**inputs) -> np.ndarray:
    y, _ = _run(inputs, trace=False)
    return y
